# revision 1
# baseline (speedup 1.0000x reference)
"""Distributed 3-layer GAT on 8 TRN2 NeuronCores (Bass/Tile).

Sharding: core c owns dst nodes [c*NS, (c+1)*NS). Edges partitioned by dst
shard, sorted by (src-chunk, dst-chunk, dst); sc-major tile numbering.
Weights replicated. Per layer: sharded dense (h @ [W|ws|wd]) -> AllGather a
bf16 512B-row gather table -> edge phase: bulk dma_gather of src rows
(4 SWDGE queues, calls merged over super-chunk pairs), host-precomputed
bf16 one-hot eq/eqT matrices drive PE matmuls that accumulate weighted
messages + softmax denominators in PSUM and expand s_dst to edges.
Normalization per node after accumulation (division commutes with the
segment sum; amax subtraction skipped: |alpha| <~ 6 so exp() is safe).
"""
import os, sys, types
sys.path.insert(0, "/opt/trn_rl_repo")
import numpy as np
import ml_dtypes

import antenv
if "antenv.axon_hooks" not in sys.modules:
    _hooks_mod = types.ModuleType("antenv.axon_hooks")
    _HOOK = [None]
    _hooks_mod.set_axon_ntff_profile_hook = lambda h: _HOOK.__setitem__(0, h)
    _hooks_mod.get_axon_ntff_profile_hook = lambda: _HOOK[0]
    sys.modules["antenv.axon_hooks"] = _hooks_mod
    antenv.axon_hooks = _hooks_mod
    try:
        import trn_agent_boot.trn_boot as _tb
        _h = _tb._ntff_profile_via_ctypes("/opt/axon/libaxon_pjrt.so")
        if _h is not None:
            _hooks_mod.set_axon_ntff_profile_hook(_h)
    except Exception:
        pass

from concourse import bass, bacc, mybir, tile
from concourse import bass_utils
from concourse.bass_utils import run_bass_kernel_spmd
from concourse._compat import get_trn_type
bass_utils.upload_artifacts = lambda tmpdir: "local://noop"

F32 = mybir.dt.float32
BF16 = mybir.dt.bfloat16
I16 = mybir.dt.int16
NPBF = ml_dtypes.bfloat16
P = 128
SLOPE = 0.2

N = 100000
IN, HID, HEADS, OUT = 128, 32, 4, 32
NCORES = 8
NS = N // NCORES
NCH = (NS + P - 1) // P
S_CH = 3
PAIR = 2
NSC = 4
CH = N // NSC
ROWB = 256
FEAT = 128
NCOL = 136


def _build_wext(W, a_s, a_d):
    Fin = W.shape[0]
    H, C = a_s.shape
    Wr = W.reshape(Fin, H, C)
    ws = np.einsum("fhc,hc->fh", Wr, a_s)
    wd = np.einsum("fhc,hc->fh", Wr, a_d)
    return np.ascontiguousarray(np.concatenate([W, ws, wd], axis=1), np.float32)


def _prep_graph(edge_index):
    loops = np.arange(N, dtype=np.int64)
    src = np.concatenate([edge_index[0].astype(np.int64), loops])
    dst = np.concatenate([edge_index[1].astype(np.int64), loops])

    core = dst // NS
    per_core = []
    counts = np.zeros((NCORES, NCH, NSC), dtype=np.int64)
    for c in range(NCORES):
        m = core == c
        s_c, d_c = src[m], dst[m]
        dl = d_c - c * NS
        ch = dl // P
        sc = s_c // CH
        order = np.lexsort((d_c, ch, sc))
        s_c, dl, ch, sc = s_c[order], dl[order], ch[order], sc[order]
        np.add.at(counts[c], (ch, sc), 1)
        per_core.append((s_c, dl, ch, sc))

    T = np.ceil(counts.max(axis=0) / P).astype(np.int64)
    tile_off = np.zeros((NCH, NSC), dtype=np.int64)
    acc = 0
    for sc in range(NSC):
        for ch in range(NCH):
            tile_off[ch, sc] = acc
            acc += T[ch, sc]
    ntiles = acc

    gidx = np.zeros((NCORES, ntiles * P), dtype=np.int16)
    dstloc = np.full((NCORES, ntiles, P), -1.0, dtype=np.float32)
    for c in range(NCORES):
        s_c, dl, ch, sc = per_core[c]
        pos = 0
        for scv in range(NSC):
            for chv in range(NCH):
                n = int(counts[c, chv, scv])
                if n == 0:
                    continue
                t0 = int(tile_off[chv, scv])
                gidx[c, t0 * P: t0 * P + n] = (s_c[pos:pos + n] - scv * CH).astype(np.int16)
                dstloc[c].reshape(-1)[t0 * P: t0 * P + n] = (dl[pos:pos + n] - chv * P).astype(np.float32)
                pos += n
        assert pos == len(s_c)
    return T, tile_off, int(ntiles), gidx, dstloc


def _wrap_idx(flat16):
    n = flat16.shape[0]
    w = flat16.reshape(n // 16, 16).T
    return np.ascontiguousarray(np.tile(w, (8, 1)), np.int16)


def _build_program(T, tile_off, ntiles):
    nc = bacc.Bacc(get_trn_type() or "TRN2", target_bir_lowering=False,
                   debug=False, enable_asserts=False, num_devices=NCORES,
                   num_swdge_queues=4)
    x_sh = nc.dram_tensor("x_shard", [NS, FEAT], F32, kind="ExternalInput").ap()
    gidx_t = nc.dram_tensor("gidx", [P, ntiles * 8], I16, kind="ExternalInput").ap()
    eq_t = nc.dram_tensor("eq", [P, ntiles * P], BF16, kind="ExternalInput").ap()
    eqT_t = nc.dram_tensor("eqT", [P, ntiles * P], BF16, kind="ExternalInput").ap()
    wext_t = [nc.dram_tensor(f"wext{l}", [FEAT, NCOL], F32, kind="ExternalInput").ap() for l in range(3)]
    btile_t = [nc.dram_tensor(f"btile{l}", [P, FEAT if l < 2 else OUT], F32, kind="ExternalInput").ap() for l in range(3)]
    ident_t = nc.dram_tensor("ident", [P, P], F32, kind="ExternalInput").ap()
    out_t = nc.dram_tensor("out", [NS, OUT], F32, kind="ExternalOutput").ap()

    groups = [list(range(NCORES))]
    NSUP = (NCH + S_CH - 1) // S_CH
    NPAIR = (NSUP + PAIR - 1) // PAIR

    with tile.TileContext(nc) as tc:
        with (
            tc.tile_pool(name="const", bufs=1) as constp,
            tc.tile_pool(name="sched", bufs=1) as schedp,
            tc.tile_pool(name="gpool", bufs=5) as gpool,
            tc.tile_pool(name="eqp", bufs=5) as eqp,
            tc.tile_pool(name="work", bufs=4) as work,
            tc.tile_pool(name="sdcp", bufs=8) as sdcp,
            tc.tile_pool(name="nodep", bufs=3) as nodep,
            tc.tile_pool(name="accp", bufs=6, space="PSUM") as accp,
            tc.tile_pool(name="tpp", bufs=1, space="PSUM") as tpp,
            tc.tile_pool(name="sdp", bufs=1, space="PSUM") as sdp,
            tc.tile_pool(name="dram", bufs=2, space="DRAM") as dramp,
        ):
            ident = constp.tile([P, P], F32, tag="ident")
            nc.sync.dma_start(out=ident[:], in_=ident_t[:])
            wext, btile = [], []
            for l in range(3):
                w = constp.tile([FEAT, NCOL], F32, tag=f"wext{l}")
                nc.sync.dma_start(out=w[:], in_=wext_t[l][:])
                wext.append(w)
                b = constp.tile([P, FEAT if l < 2 else OUT], F32, tag=f"bt{l}")
                nc.sync.dma_start(out=b[:], in_=btile_t[l][:])
                btile.append(b)
            gidx_sb = schedp.tile([P, ntiles * 8], I16, tag="gidx")
            nc.sync.dma_start(out=gidx_sb[:], in_=gidx_t[:])

            xwss_sh = [dramp.tile([NS, ROWB], BF16, tag="xwsh", name=f"xwsh{i}") for i in range(3)]
            xwss_full = [dramp.tile([N, ROWB], BF16, tag="xwfull", name=f"xwfull{i}", addr_space="Shared") for i in range(3)]
            sd_sh = [dramp.tile([NS, 4], BF16, tag="sdsh", name=f"sdsh{i}") for i in range(3)]

            def dense_tile(h_sb, lnext, base, nn):
                hT_ps = tpp.tile([P, P], F32, tag="tp")
                nc.tensor.transpose(out=hT_ps[:], in_=h_sb[:], identity=ident[:])
                hT_sb = work.tile([P, P], F32, tag="hT")
                nc.scalar.copy(out=hT_sb[:], in_=hT_ps[:])
                d_ps = tpp.tile([P, NCOL], F32, tag="tp", name="d_ps")
                nc.tensor.matmul(out=d_ps[:], lhsT=hT_sb[:], rhs=wext[lnext][:],
                                 start=True, stop=True)
                xo = work.tile([P, NCOL], BF16, tag="xo")
                nc.scalar.copy(out=xo[:], in_=d_ps[:])
                nc.sync.dma_start(out=xwss_sh[lnext][base:base + nn, 0:NCOL],
                                  in_=xo[:nn, :])
                nc.sync.dma_start(out=sd_sh[lnext][base:base + nn, :],
                                  in_=xo[:nn, 132:136])

            for t in range(NCH):
                base = t * P
                nn = min(P, NS - base)
                xt = work.tile([P, FEAT], F32, tag="xt")
                nc.sync.dma_start(out=xt[:nn, :], in_=x_sh[base:base + nn, :])
                dense_tile(xt, 0, base, nn)

            for layer in range(3):
                nc.gpsimd.collective_compute(
                    "AllGather", mybir.AluOpType.bypass, replica_groups=groups,
                    ins=[xwss_sh[layer].opt()], outs=[xwss_full[layer].opt()])

                for pair in range(NPAIR):
                    sup0 = pair * PAIR
                    ch0 = sup0 * S_CH
                    chn = min(PAIR * S_CH, NCH - ch0)

                    gouts, eqgs, eqTgs, spans = [], [], [], []
                    for sc in range(NSC):
                        t0 = int(tile_off[ch0, sc])
                        ncall = int(sum(T[ch0 + ci, sc] for ci in range(chn)))
                        spans.append((t0, ncall))
                        if ncall == 0:
                            gouts.append(None); eqgs.append(None); eqTgs.append(None)
                            continue
                        gout = gpool.tile([P, ncall, ROWB], BF16, tag="g")
                        nc.gpsimd.dma_gather(
                            out_ap=gout[:],
                            in_ap=xwss_full[layer][sc * CH:(sc + 1) * CH, :],
                            idxs_ap=gidx_sb[:, t0 * 8:(t0 + ncall) * 8],
                            num_idxs=ncall * P, num_idxs_reg=ncall * P,
                            elem_size=ROWB, single_packet=False, queue_num=sc % 4)
                        eqg = eqp.tile([P, ncall, P], BF16, tag="eq")
                        nc.sync.dma_start(out=eqg[:], in_=eq_t[:, t0 * P:(t0 + ncall) * P])
                        eqTg = eqp.tile([P, ncall, P], BF16, tag="eqT")
                        nc.sync.dma_start(out=eqTg[:], in_=eqT_t[:, t0 * P:(t0 + ncall) * P])
                        gouts.append(gout); eqgs.append(eqg); eqTgs.append(eqTg)

                    sdch = []
                    for ci in range(chn):
                        base = (ch0 + ci) * P
                        nn = min(P, NS - base)
                        s = sdcp.tile([P, 4], BF16, tag="sd")
                        nc.sync.dma_start(out=s[:nn, :], in_=sd_sh[layer][base:base + nn, :])
                        sdch.append(s)

                    acc_ps = [accp.tile([P, NCOL], F32, tag="acc",
                                        name=f"acc_l{layer}p{pair}c{ci}") for ci in range(chn)]
                    mm_count = [0] * chn
                    mm_total = [int(T[ch0 + ci, :].sum()) for ci in range(chn)]

                    for sc in range(NSC):
                        t0, ncall = spans[sc]
                        if ncall == 0:
                            continue
                        gout, eqg, eqTg = gouts[sc], eqgs[sc], eqTgs[sc]
                        sd_ps = sdp.tile([P, ncall, 4], F32, tag="sdps")
                        tlist = []
                        slot = 0
                        for ci in range(chn):
                            for _ in range(int(T[ch0 + ci, sc])):
                                nc.tensor.matmul(out=sd_ps[:, slot, :],
                                                 lhsT=eqTg[:, slot, :],
                                                 rhs=sdch[ci][:], start=True, stop=True)
                                tlist.append(ci)
                                slot += 1

                        al = work.tile([P, ncall, 4], F32, tag="al")
                        nc.vector.tensor_tensor(out=al[:], in0=gout[:, :, 128:132],
                                                in1=sd_ps[:], op=mybir.AluOpType.add)
                        al2 = work.tile([P, ncall, 4], F32, tag="al2")
                        nc.vector.tensor_scalar(out=al2[:], in0=al[:], scalar1=SLOPE,
                                                scalar2=None, op0=mybir.AluOpType.mult)
                        nc.vector.tensor_tensor(out=al[:], in0=al[:], in1=al2[:],
                                                op=mybir.AluOpType.max)
                        nc.scalar.activation(out=gout[:, :, 132:136], in_=al[:],
                                             func=mybir.ActivationFunctionType.Exp)
                        nc.vector.tensor_tensor(
                            out=gout[:, :, 0:128].rearrange("p t (h c) -> p t h c", h=4),
                            in0=gout[:, :, 0:128].rearrange("p t (h c) -> p t h c", h=4),
                            in1=gout[:, :, 132:136].unsqueeze(3).broadcast_to([P, ncall, 4, 32]),
                            op=mybir.AluOpType.mult)

                        for slot, ci in enumerate(tlist):
                            mm_count[ci] += 1
                            nc.tensor.matmul(
                                out=acc_ps[ci][:], lhsT=eqg[:, slot, :],
                                rhs=gout[:, slot, 0:NCOL],
                                start=(mm_count[ci] == 1),
                                stop=(mm_count[ci] == mm_total[ci]))

                    for ci in range(chn):
                        base = (ch0 + ci) * P
                        nn = min(P, NS - base)
                        r = work.tile([P, 4], F32, tag="r")
                        nc.vector.reciprocal(out=r[:], in_=acc_ps[ci][:, 132:136])
                        h = nodep.tile([P, FEAT], F32, tag="h")
                        nc.vector.tensor_tensor(
                            out=h[:].rearrange("p (h c) -> p h c", h=4),
                            in0=acc_ps[ci][:, 0:128].rearrange("p (h c) -> p h c", h=4),
                            in1=r[:].unsqueeze(2).broadcast_to([P, 4, 32]),
                            op=mybir.AluOpType.mult)
                        if layer < 2:
                            nc.vector.tensor_tensor(out=h[:], in0=h[:], in1=btile[layer][:],
                                                    op=mybir.AluOpType.add)
                            mn = nodep.tile([P, FEAT], F32, tag="mn")
                            nc.vector.tensor_scalar(out=mn[:], in0=h[:], scalar1=0.0,
                                                    scalar2=None, op0=mybir.AluOpType.min)
                            nc.scalar.activation(out=mn[:], in_=mn[:],
                                                 func=mybir.ActivationFunctionType.Exp)
                            nc.vector.tensor_scalar(out=h[:], in0=h[:], scalar1=0.0,
                                                    scalar2=-1.0, op0=mybir.AluOpType.max,
                                                    op1=mybir.AluOpType.add)
                            nc.vector.tensor_tensor(out=h[:], in0=h[:], in1=mn[:],
                                                    op=mybir.AluOpType.add)
                            dense_tile(h, layer + 1, base, nn)
                        else:
                            o = nodep.tile([P, OUT], F32, tag="o")
                            hv = h[:].rearrange("p (h c) -> p h c", h=4)
                            nc.vector.tensor_tensor(out=o[:], in0=hv[:, 0, :], in1=hv[:, 1, :],
                                                    op=mybir.AluOpType.add)
                            nc.vector.tensor_tensor(out=o[:], in0=o[:], in1=hv[:, 2, :],
                                                    op=mybir.AluOpType.add)
                            nc.vector.tensor_tensor(out=o[:], in0=o[:], in1=hv[:, 3, :],
                                                    op=mybir.AluOpType.add)
                            nc.vector.tensor_scalar(out=o[:], in0=o[:], scalar1=0.25,
                                                    scalar2=None, op0=mybir.AluOpType.mult)
                            nc.vector.tensor_tensor(out=o[:], in0=o[:], in1=btile[2][:],
                                                    op=mybir.AluOpType.add)
                            nc.sync.dma_start(out=out_t[base:base + nn, :], in_=o[:nn, :])
    nc.compile()
    return nc


def kernel(x, edge_index, W1, as1, ad1, b1, W2, as2, ad2, b2, W3, as3, ad3, b3):
    x = np.asarray(x, np.float32)
    edge_index = np.asarray(edge_index)
    T, tile_off, ntiles, gidx, dstloc = _prep_graph(edge_index)
    nc = _build_program(T, tile_off, ntiles)

    wext = [_build_wext(np.asarray(W1, np.float32), np.asarray(as1, np.float32), np.asarray(ad1, np.float32)),
            _build_wext(np.asarray(W2, np.float32), np.asarray(as2, np.float32), np.asarray(ad2, np.float32)),
            _build_wext(np.asarray(W3, np.float32), np.asarray(as3, np.float32), np.asarray(ad3, np.float32))]
    bt = [np.ascontiguousarray(np.tile(np.asarray(b, np.float32)[None, :], (P, 1)))
          for b in (b1, b2, b3)]
    ident_np = np.eye(P, dtype=np.float32)
    jj = np.arange(P, dtype=np.float32)

    in_maps = []
    for c in range(NCORES):
        dl = dstloc[c]
        eq_full = (dl[:, :, None] == jj[None, None, :])
        eq_np = np.ascontiguousarray(
            eq_full.transpose(1, 0, 2).reshape(P, ntiles * P)).astype(NPBF)
        eqT_np = np.ascontiguousarray(
            eq_full.transpose(2, 0, 1).reshape(P, ntiles * P)).astype(NPBF)
        m = {
            "x_shard": np.ascontiguousarray(x[c * NS:(c + 1) * NS]),
            "gidx": _wrap_idx(gidx[c]),
            "eq": eq_np, "eqT": eqT_np,
            "ident": ident_np,
        }
        for l in range(3):
            m[f"wext{l}"] = wext[l]
            m[f"btile{l}"] = bt[l]
        in_maps.append(m)

    trace = bool(int(os.environ.get("GAT_TRACE", "0")))
    res = run_bass_kernel_spmd(nc, in_maps, list(range(NCORES)), trace=trace)
    kernel.last_exec_time_ns = res.exec_time_ns
    out = np.concatenate([res.results[c]["out"] for c in range(NCORES)], axis=0)
    return out


kernel.last_exec_time_ns = None



# revision 17
# speedup vs baseline: 1.4118x; 1.4118x over previous
"""Distributed 3-layer GAT on 8 TRN2 NeuronCores (Bass/Tile).

Sharding: core c owns dst nodes [c*NS, (c+1)*NS). Within each core, dst
nodes are re-packed into 128-node chunks by a balanced vector-bin-packing
so that per-(chunk, src-window) edge counts align to multiples of 128
(minimizes gather-tile padding; ~8% vs ~45% naive). Self-loops are removed
from the edge list and folded in analytically during normalization.
Weights replicated. Per layer: sharded dense (h @ [W|ws|wd]) -> AllGather a
bf16 512B-row gather table -> edge phase: bulk dma_gather of src rows
(4 SWDGE queues), one-hot eq matrices GENERATED ON DEVICE (DVE is_equal
against iota for the edge->dst map; PE ones-outer-product broadcast + DVE
is_equal for its transpose), PE matmuls accumulate weighted messages +
softmax denominators in PSUM; s_dst expanded to edges via eqT matmul and
s_src added via an identity matmul into the same PSUM accumulator.
Normalization per node after accumulation (division commutes with the
segment sum; amax subtraction skipped: |alpha| <~ 6 so exp() is safe).
Host un-permutes the packed output rows.
"""
import os, sys, types
sys.path.insert(0, "/opt/trn_rl_repo")
import numpy as np
import ml_dtypes

import antenv
if "antenv.axon_hooks" not in sys.modules:
    _hooks_mod = types.ModuleType("antenv.axon_hooks")
    _HOOK = [None]
    _hooks_mod.set_axon_ntff_profile_hook = lambda h: _HOOK.__setitem__(0, h)
    _hooks_mod.get_axon_ntff_profile_hook = lambda: _HOOK[0]
    sys.modules["antenv.axon_hooks"] = _hooks_mod
    antenv.axon_hooks = _hooks_mod
    try:
        import trn_agent_boot.trn_boot as _tb
        _h = _tb._ntff_profile_via_ctypes("/opt/axon/libaxon_pjrt.so")
        if _h is not None:
            _hooks_mod.set_axon_ntff_profile_hook(_h)
    except Exception:
        pass

from concourse import bass, bacc, mybir, tile
from concourse import bass_utils
from concourse.bass_utils import run_bass_kernel_spmd
from concourse._compat import get_trn_type
bass_utils.upload_artifacts = lambda tmpdir: "local://noop"

F32 = mybir.dt.float32
BF16 = mybir.dt.bfloat16
I16 = mybir.dt.int16
NPBF = ml_dtypes.bfloat16
P = 128
SLOPE = 0.2

N = 100000
IN, HID, HEADS, OUT = 128, 32, 4, 32
NCORES = 8
NS = N // NCORES
NCH = (NS + P - 1) // P
S_CH = 3
PAIR = 2
NSC = 4
CH = N // NSC
ROWB = 256
FEAT = 128
NCOL = 136
ACOL = 132
NSUP = (NCH + S_CH - 1) // S_CH
NPAIR = (NSUP + PAIR - 1) // PAIR
NSPAN = NPAIR * NSC


def _build_wext(W, a_s, a_d):
    Fin = W.shape[0]
    H, C = a_s.shape
    Wr = W.reshape(Fin, H, C)
    ws = np.einsum("fhc,hc->fh", Wr, a_s)
    wd = np.einsum("fhc,hc->fh", Wr, a_d)
    return np.ascontiguousarray(np.concatenate([W, ws, wd], axis=1), np.float32)


def _pack_core(v, caps, nslots):
    """Greedy balanced vector bin packing: assign dsts (rows of v, [NS, NSC]
    per-window edge counts) to NCH chunks of <=128 dsts, keeping per-(chunk,
    window) fills under caps."""
    order = np.argsort(-v.sum(1), kind="stable")
    fill = np.zeros((NCH, NSC), dtype=np.float64)
    slots = nslots.copy()
    assign = np.empty(v.shape[0], dtype=np.int64)
    capsf = caps.astype(np.float64)
    for d in order:
        need = v[d]
        newfill = fill + need
        frac = (newfill / capsf).max(axis=1)
        score = frac + (slots <= 0) * 1e9 + np.any(newfill > capsf, axis=1) * 1e3
        b = int(np.argmin(score))
        assign[d] = b
        fill[b] += need
        slots[b] -= 1
    return assign, fill.astype(np.int64)


def _wrap_idx(flat16):
    n = flat16.shape[0]
    w = flat16.reshape(n // 16, 16).T
    return np.ascontiguousarray(np.tile(w, (8, 1)), np.int16)


def _prep_graph(edge_index):
    src = edge_index[0].astype(np.int64)
    dst = edge_index[1].astype(np.int64)
    core = dst // NS
    sc_of_src = (src // NS) // (CH // NS)  # window of src (perm-invariant)

    caps = np.full((NCH, NSC), 3, dtype=np.int64)
    for ch in range(NCH):
        caps[ch, ch % NSC] = 4
    caps *= P
    nslots = np.full(NCH, P, dtype=np.int64)
    nslots[-1] = NS - (NCH - 1) * P

    newls, fills, edges = [], [], []
    for c in range(NCORES):
        m = core == c
        s_c, d_c = src[m], dst[m]
        dl = d_c - c * NS
        sc_e = sc_of_src[m]
        v = np.zeros((NS, NSC), dtype=np.int64)
        np.add.at(v, (dl, sc_e), 1)
        assign, fill = _pack_core(v, caps, nslots)
        # position within chunk: members in ascending dst order
        newl = np.empty(NS, dtype=np.int64)
        for ch in range(NCH):
            mem = np.where(assign == ch)[0]
            newl[mem] = ch * P + np.arange(len(mem))
        newls.append(newl)
        fills.append(fill)
        edges.append((s_c, dl, sc_e))

    fills = np.array(fills)
    T = np.ceil(fills.max(axis=0) / P).astype(np.int64)
    tile_off = np.zeros((NCH, NSC), dtype=np.int64)
    acc = 0
    for sc in range(NSC):
        for ch in range(NCH):
            tile_off[ch, sc] = acc
            acc += T[ch, sc]
    ntiles = int(acc)

    spans = []
    maxspan = 0
    for pair in range(NPAIR):
        ch0 = pair * PAIR * S_CH
        chn = min(PAIR * S_CH, NCH - ch0)
        for sc in range(NSC):
            t0 = int(tile_off[ch0, sc])
            ncall = int(sum(T[ch0 + ci, sc] for ci in range(chn)))
            spans.append((t0, ncall))
            maxspan = max(maxspan, ncall)

    gidx = np.zeros((NCORES, ntiles * P), dtype=np.int16)
    dloc = np.full((NCORES, ntiles, P), -1.0, dtype=np.float32)
    for c in range(NCORES):
        s_c, dl, sc_e = edges[c]
        newl = newls[c]
        ch_e = newl[dl] // P
        dpos = newl[dl] - ch_e * P
        cs = s_c // NS
        all_newl = np.stack(newls)  # [NCORES, NS]
        snew = cs * NS + all_newl[cs, s_c - cs * NS]
        gval = (snew - sc_e * CH).astype(np.int16)
        order = np.lexsort((ch_e, sc_e))
        s_o, ch_o, sc_o, gv_o, dp_o = (
            s_c[order], ch_e[order], sc_e[order], gval[order], dpos[order])
        # bucket boundaries in (sc, ch) order
        key = sc_o * NCH + ch_o
        uniq, starts = np.unique(key, return_index=True)
        starts = list(starts) + [len(key)]
        for i, k in enumerate(uniq):
            sc_b, ch_b = int(k) // NCH, int(k) % NCH
            a, b = starts[i], starts[i + 1]
            n = b - a
            t0 = int(tile_off[ch_b, sc_b])
            assert n <= int(T[ch_b, sc_b]) * P, (c, ch_b, sc_b, n)
            pos = t0 * P + np.arange(n)
            gidx[c][pos] = gv_o[a:b]
            dloc[c].reshape(-1)[pos] = dp_o[a:b]

    return T, tile_off, ntiles, spans, maxspan, gidx, dloc, newls


def _build_program(T, tile_off, ntiles, spans, maxspan):
    nc = bacc.Bacc(get_trn_type() or "TRN2", target_bir_lowering=False,
                   debug=False, enable_asserts=False, num_devices=NCORES,
                   num_swdge_queues=4)
    x_sh = nc.dram_tensor("x_shard", [NS, FEAT], F32, kind="ExternalInput").ap()
    gidx_t = nc.dram_tensor("gidx", [P, ntiles * 8], I16, kind="ExternalInput").ap()
    dlocT_t = nc.dram_tensor("dlocT", [P, ntiles], BF16, kind="ExternalInput").ap()
    dspan_t = nc.dram_tensor("dspan", [NSPAN, maxspan * P], BF16, kind="ExternalInput").ap()
    iota_r_t = nc.dram_tensor("iota_r", [P, P], BF16, kind="ExternalInput").ap()
    iota_c_t = nc.dram_tensor("iota_c", [P, 1], F32, kind="ExternalInput").ap()
    ones_r_t = nc.dram_tensor("ones_r", [1, P], BF16, kind="ExternalInput").ap()
    identb_t = nc.dram_tensor("identb", [P, P], BF16, kind="ExternalInput").ap()
    wext_t = [nc.dram_tensor(f"wext{l}", [FEAT, NCOL], F32, kind="ExternalInput").ap() for l in range(3)]
    btile_t = [nc.dram_tensor(f"btile{l}", [P, FEAT if l < 2 else OUT], F32, kind="ExternalInput").ap() for l in range(3)]
    ident_t = nc.dram_tensor("ident", [P, P], F32, kind="ExternalInput").ap()
    out_t = nc.dram_tensor("out", [NS, OUT], F32, kind="ExternalOutput").ap()

    groups = [list(range(NCORES))]

    with tile.TileContext(nc) as tc:
        with (
            tc.tile_pool(name="const", bufs=1) as constp,
            tc.tile_pool(name="sched", bufs=1) as schedp,
            tc.tile_pool(name="gpool", bufs=5) as gpool,
            tc.tile_pool(name="eqp", bufs=4) as eqp,
            tc.tile_pool(name="eqtp", bufs=4) as eqtp,
            tc.tile_pool(name="gwp", bufs=4) as gwp,
            tc.tile_pool(name="work", bufs=4) as work,
            tc.tile_pool(name="selfp", bufs=8) as selfp,
            tc.tile_pool(name="nodep", bufs=3) as nodep,
            tc.tile_pool(name="accp", bufs=6, space="PSUM") as accp,
            tc.tile_pool(name="tpp", bufs=1, space="PSUM") as tpp,
            tc.tile_pool(name="sdp", bufs=1, space="PSUM") as sdp,
            tc.tile_pool(name="dram", bufs=2, space="DRAM") as dramp,
        ):
            ident = constp.tile([P, P], F32, tag="ident")
            nc.sync.dma_start(out=ident[:], in_=ident_t[:])
            identb = constp.tile([P, P], BF16, tag="identb")
            nc.sync.dma_start(out=identb[:], in_=identb_t[:])
            iota_r = constp.tile([P, P], BF16, tag="iota_r")
            nc.sync.dma_start(out=iota_r[:], in_=iota_r_t[:])
            iota_c = constp.tile([P, 1], F32, tag="iota_c")
            nc.sync.dma_start(out=iota_c[:], in_=iota_c_t[:])
            ones_r = constp.tile([1, P], BF16, tag="ones_r")
            nc.sync.dma_start(out=ones_r[:], in_=ones_r_t[:])
            wext, btile = [], []
            for l in range(3):
                w = constp.tile([FEAT, NCOL], F32, tag=f"wext{l}")
                nc.sync.dma_start(out=w[:], in_=wext_t[l][:])
                wext.append(w)
                b = constp.tile([P, FEAT if l < 2 else OUT], F32, tag=f"bt{l}")
                nc.sync.dma_start(out=b[:], in_=btile_t[l][:])
                btile.append(b)
            gidx_sb = schedp.tile([P, ntiles * 8], I16, tag="gidx")
            nc.sync.dma_start(out=gidx_sb[:], in_=gidx_t[:])
            dlocT_sb = schedp.tile([P, ntiles], BF16, tag="dlocT")
            nc.sync.dma_start(out=dlocT_sb[:], in_=dlocT_t[:])

            xwss_sh = [dramp.tile([NS, ROWB], BF16, tag="xwsh", name=f"xwsh{i}") for i in range(3)]
            xwss_full = [dramp.tile([N, ROWB], BF16, tag="xwfull", name=f"xwfull{i}", addr_space="Shared") for i in range(3)]

            def dense_tile(h_sb, lnext, base, nn):
                hT_ps = tpp.tile([P, P], F32, tag="tp")
                nc.tensor.transpose(out=hT_ps[:], in_=h_sb[:], identity=ident[:])
                hT_sb = work.tile([P, P], F32, tag="hT")
                nc.scalar.copy(out=hT_sb[:], in_=hT_ps[:])
                d_ps = tpp.tile([P, NCOL], F32, tag="tp", name="d_ps")
                nc.tensor.matmul(out=d_ps[:], lhsT=hT_sb[:], rhs=wext[lnext][:],
                                 start=True, stop=True)
                xo = work.tile([P, NCOL], BF16, tag="xo")
                nc.scalar.copy(out=xo[:], in_=d_ps[:])
                nc.sync.dma_start(out=xwss_sh[lnext][base:base + nn, 0:NCOL],
                                  in_=xo[:nn, :])

            for t in range(NCH):
                base = t * P
                nn = min(P, NS - base)
                xt = work.tile([P, FEAT], F32, tag="xt")
                nc.sync.dma_start(out=xt[:nn, :], in_=x_sh[base:base + nn, :])
                dense_tile(xt, 0, base, nn)

            for layer in range(3):
                nc.gpsimd.collective_compute(
                    "AllGather", mybir.AluOpType.bypass, replica_groups=groups,
                    ins=[xwss_sh[layer].opt()], outs=[xwss_full[layer].opt()])

                for pair in range(NPAIR):
                    ch0 = pair * PAIR * S_CH
                    chn = min(PAIR * S_CH, NCH - ch0)

                    # gathers first (gpsimd runs ahead, gated only by bufs)
                    gouts = []
                    for sc in range(NSC):
                        t0, ncall = spans[pair * NSC + sc]
                        if ncall == 0:
                            gouts.append(None)
                            continue
                        gout = gpool.tile([P, ncall, ROWB], BF16, tag="g")
                        nc.gpsimd.dma_gather(
                            out_ap=gout[:],
                            in_ap=xwss_full[layer][sc * CH:(sc + 1) * CH, :],
                            idxs_ap=gidx_sb[:, t0 * 8:(t0 + ncall) * 8],
                            num_idxs=ncall * P, num_idxs_reg=ncall * P,
                            elem_size=ROWB, single_packet=False, queue_num=sc % 4)
                        gouts.append(gout)

                    # local per-chunk rows (for s_dst table + self-loop term)
                    xws = []
                    for ci in range(chn):
                        base = (ch0 + ci) * P
                        nn = min(P, NS - base)
                        s = selfp.tile([P, NCOL], BF16, tag="xws")
                        nc.sync.dma_start(out=s[:nn, :],
                                          in_=xwss_sh[layer][base:base + nn, 0:NCOL])
                        xws.append(s)

                    # phase A (gather-independent): eq gen
                    sd_all = sdp.tile([P, NSC, maxspan, 4], F32, tag="sdps",
                                      name=f"sd_l{layer}p{pair}")
                    eqgs, eqTs = [], []
                    for sc in range(NSC):
                        t0, ncall = spans[pair * NSC + sc]
                        if ncall == 0:
                            eqgs.append(None)
                            eqTs.append(None)
                            continue
                        eqg = eqp.tile([P, ncall, P], BF16, tag="eqg")
                        nc.vector.tensor_tensor(
                            out=eqg[:],
                            in0=dlocT_sb[:, t0:t0 + ncall].unsqueeze(2).broadcast_to([P, ncall, P]),
                            in1=iota_r[:].unsqueeze(1).broadcast_to([P, ncall, P]),
                            op=mybir.AluOpType.is_equal)
                        eqgs.append(eqg)
                        eqT = eqtp.tile([P, ncall, P], BF16, tag="eqT")
                        sidx = pair * NSC + sc
                        drow = work.tile([1, maxspan * P], BF16, tag="drow")
                        nc.sync.dma_start(out=drow[:, 0:ncall * P],
                                          in_=dspan_t[sidx:sidx + 1, 0:ncall * P])
                        off = 0
                        for ci in range(chn):
                            tci = int(T[ch0 + ci, sc])
                            if tci == 0:
                                continue
                            B = tpp.tile([P, 4, P], F32, tag="tp", name="bcast")
                            nc.tensor.matmul(
                                out=B[:, 0:tci, :], lhsT=ones_r[:],
                                rhs=drow[0:1, off * P:(off + tci) * P],
                                start=True, stop=True)
                            nc.vector.tensor_tensor(
                                out=eqT[:, off:off + tci, :],
                                in0=B[:, 0:tci, :],
                                in1=iota_c[:].unsqueeze(2).broadcast_to([P, tci, P]),
                                op=mybir.AluOpType.is_equal)
                            off += tci
                        eqTs.append(eqT)

                    acc_tiles = [accp.tile([P, ACOL], F32, tag="acc",
                                           name=f"acc_l{layer}p{pair}c{k}") for k in range(chn)]

                    def acc_sl(ci, lo=0, hi=ACOL):
                        return acc_tiles[ci][:, lo:hi]
                    mm_count = [0] * chn
                    mm_total = [int(T[ch0 + ci, :].sum()) for ci in range(chn)]

                    # phase B (gather-dependent)
                    for sc in range(NSC):
                        t0, ncall = spans[pair * NSC + sc]
                        if ncall == 0:
                            continue
                        gout, eqg, eqT = gouts[sc], eqgs[sc], eqTs[sc]
                        sd_ps = sd_all[:, sc, 0:ncall, :]
                        slot = 0
                        for ci in range(chn):
                            for _ in range(int(T[ch0 + ci, sc])):
                                nc.tensor.matmul(out=sd_all[:, sc, slot, :],
                                                 lhsT=eqT[:, slot, :],
                                                 rhs=xws[ci][:, 132:136],
                                                 start=True, stop=False)
                                nc.tensor.matmul(out=sd_all[:, sc, slot, :],
                                                 lhsT=identb[:],
                                                 rhs=gout[:, slot, 128:132],
                                                 start=False, stop=True)
                                slot += 1
                        al2 = work.tile([P, ncall, 4], F32, tag="al2")
                        nc.vector.tensor_scalar(out=al2[:], in0=sd_ps[:], scalar1=SLOPE,
                                                scalar2=None, op0=mybir.AluOpType.mult)
                        al = work.tile([P, ncall, 4], F32, tag="al")
                        nc.vector.tensor_tensor(out=al[:], in0=sd_ps[:], in1=al2[:],
                                                op=mybir.AluOpType.max)
                        exb = work.tile([P, ncall, 4], BF16, tag="exb")
                        nc.scalar.activation(out=exb[:], in_=al[:],
                                             func=mybir.ActivationFunctionType.Exp)
                        gw = gwp.tile([P, ncall, ACOL], BF16, tag="gw")
                        nc.vector.tensor_tensor(
                            out=gw[:, :, 0:128].rearrange("p t (h c) -> p t h c", h=4),
                            in0=gout[:, :, 0:128].rearrange("p t (h c) -> p t h c", h=4),
                            in1=exb[:].unsqueeze(3).broadcast_to([P, ncall, 4, 32]),
                            op=mybir.AluOpType.mult)
                        nc.scalar.copy(out=gw[:, :, 128:132], in_=exb[:])
                        slot = 0
                        for ci in range(chn):
                            for _ in range(int(T[ch0 + ci, sc])):
                                mm_count[ci] += 1
                                nc.tensor.matmul(
                                    out=acc_sl(ci), lhsT=eqg[:, slot, :],
                                    rhs=gw[:, slot, :],
                                    start=(mm_count[ci] == 1),
                                    stop=(mm_count[ci] == mm_total[ci]))
                                slot += 1

                    # normalize + self-loop term + next dense / output
                    for ci in range(chn):
                        base = (ch0 + ci) * P
                        nn = min(P, NS - base)
                        asf = work.tile([P, 4], F32, tag="asf")
                        nc.vector.tensor_tensor(out=asf[:], in0=xws[ci][:, 128:132],
                                                in1=xws[ci][:, 132:136],
                                                op=mybir.AluOpType.add)
                        asl = work.tile([P, 4], F32, tag="asl")
                        nc.vector.tensor_scalar(out=asl[:], in0=asf[:], scalar1=SLOPE,
                                                scalar2=None, op0=mybir.AluOpType.mult)
                        nc.vector.tensor_tensor(out=asf[:], in0=asf[:], in1=asl[:],
                                                op=mybir.AluOpType.max)
                        exs = work.tile([P, 4], F32, tag="exs")
                        nc.scalar.activation(out=exs[:], in_=asf[:],
                                             func=mybir.ActivationFunctionType.Exp)
                        den = work.tile([P, 4], F32, tag="den")
                        nc.vector.tensor_tensor(out=den[:], in0=acc_sl(ci, 128, 132),
                                                in1=exs[:], op=mybir.AluOpType.add)
                        r = work.tile([P, 4], F32, tag="r")
                        nc.vector.reciprocal(out=r[:], in_=den[:])
                        selfw = nodep.tile([P, FEAT], F32, tag="selfw")
                        nc.vector.tensor_tensor(
                            out=selfw[:].rearrange("p (h c) -> p h c", h=4),
                            in0=xws[ci][:, 0:128].rearrange("p (h c) -> p h c", h=4),
                            in1=exs[:].unsqueeze(2).broadcast_to([P, 4, 32]),
                            op=mybir.AluOpType.mult)
                        h = nodep.tile([P, FEAT], F32, tag="h")
                        nc.vector.tensor_tensor(out=h[:], in0=acc_sl(ci, 0, 128),
                                                in1=selfw[:], op=mybir.AluOpType.add)
                        nc.vector.tensor_tensor(
                            out=h[:].rearrange("p (h c) -> p h c", h=4),
                            in0=h[:].rearrange("p (h c) -> p h c", h=4),
                            in1=r[:].unsqueeze(2).broadcast_to([P, 4, 32]),
                            op=mybir.AluOpType.mult)
                        if layer < 2:
                            nc.vector.tensor_tensor(out=h[:], in0=h[:], in1=btile[layer][:],
                                                    op=mybir.AluOpType.add)
                            mn = nodep.tile([P, FEAT], F32, tag="mn")
                            nc.vector.tensor_scalar(out=mn[:], in0=h[:], scalar1=0.0,
                                                    scalar2=None, op0=mybir.AluOpType.min)
                            nc.scalar.activation(out=mn[:], in_=mn[:],
                                                 func=mybir.ActivationFunctionType.Exp)
                            nc.vector.tensor_scalar(out=h[:], in0=h[:], scalar1=0.0,
                                                    scalar2=-1.0, op0=mybir.AluOpType.max,
                                                    op1=mybir.AluOpType.add)
                            nc.vector.tensor_tensor(out=h[:], in0=h[:], in1=mn[:],
                                                    op=mybir.AluOpType.add)
                            dense_tile(h, layer + 1, base, nn)
                        else:
                            o = nodep.tile([P, OUT], F32, tag="o")
                            hv = h[:].rearrange("p (h c) -> p h c", h=4)
                            nc.vector.tensor_tensor(out=o[:], in0=hv[:, 0, :], in1=hv[:, 1, :],
                                                    op=mybir.AluOpType.add)
                            nc.vector.tensor_tensor(out=o[:], in0=o[:], in1=hv[:, 2, :],
                                                    op=mybir.AluOpType.add)
                            nc.vector.tensor_tensor(out=o[:], in0=o[:], in1=hv[:, 3, :],
                                                    op=mybir.AluOpType.add)
                            nc.vector.tensor_scalar(out=o[:], in0=o[:], scalar1=0.25,
                                                    scalar2=None, op0=mybir.AluOpType.mult)
                            nc.vector.tensor_tensor(out=o[:], in0=o[:], in1=btile[2][:],
                                                    op=mybir.AluOpType.add)
                            nc.sync.dma_start(out=out_t[base:base + nn, :], in_=o[:nn, :])
    nc.compile()
    return nc


def kernel(x, edge_index, W1, as1, ad1, b1, W2, as2, ad2, b2, W3, as3, ad3, b3):
    x = np.asarray(x, np.float32)
    edge_index = np.asarray(edge_index)
    T, tile_off, ntiles, spans, maxspan, gidx, dloc, newls = _prep_graph(edge_index)
    nc = _build_program(T, tile_off, ntiles, spans, maxspan)

    wext = [_build_wext(np.asarray(W1, np.float32), np.asarray(as1, np.float32), np.asarray(ad1, np.float32)),
            _build_wext(np.asarray(W2, np.float32), np.asarray(as2, np.float32), np.asarray(ad2, np.float32)),
            _build_wext(np.asarray(W3, np.float32), np.asarray(as3, np.float32), np.asarray(ad3, np.float32))]
    bt = [np.ascontiguousarray(np.tile(np.asarray(b, np.float32)[None, :], (P, 1)))
          for b in (b1, b2, b3)]
    ident_np = np.eye(P, dtype=np.float32)
    identb_np = np.eye(P, dtype=NPBF)
    iota_r_np = np.ascontiguousarray(
        np.tile(np.arange(P, dtype=np.float32)[None, :], (P, 1)).astype(NPBF))
    iota_c_np = np.ascontiguousarray(np.arange(P, dtype=np.float32)[:, None])
    ones_r_np = np.ones((1, P), dtype=NPBF)

    in_maps = []
    for c in range(NCORES):
        invl = np.argsort(newls[c])
        dlocT_np = np.ascontiguousarray(
            dloc[c].reshape(ntiles, P).T.astype(NPBF))
        dspan_np = np.full((NSPAN, maxspan * P), -1.0, dtype=NPBF)
        for si, (t0, ncall) in enumerate(spans):
            if ncall:
                dspan_np[si, :ncall * P] = dloc[c][t0:t0 + ncall].reshape(-1).astype(NPBF)
        m = {
            "x_shard": np.ascontiguousarray(x[c * NS:(c + 1) * NS][invl]),
            "gidx": _wrap_idx(gidx[c]),
            "dlocT": dlocT_np,
            "dspan": dspan_np,
            "iota_r": iota_r_np,
            "iota_c": iota_c_np,
            "ones_r": ones_r_np,
            "identb": identb_np,
            "ident": ident_np,
        }
        for l in range(3):
            m[f"wext{l}"] = wext[l]
            m[f"btile{l}"] = bt[l]
        in_maps.append(m)

    trace = bool(int(os.environ.get("GAT_TRACE", "0")))
    res = run_bass_kernel_spmd(nc, in_maps, list(range(NCORES)), trace=trace)
    kernel.last_exec_time_ns = res.exec_time_ns
    out = np.concatenate(
        [res.results[c]["out"][newls[c]] for c in range(NCORES)], axis=0)
    return out


kernel.last_exec_time_ns = None
